# revision 4
# baseline (speedup 1.0000x reference)
"""Trainium2 Bass kernel for nn_CrossAttention2D_ROPE (B=8, S1=4096, S2=256,
QDIM=1024, KDIM=2048, NH=16, HD=64).

Strategy: data-parallel over batch (8 cores, one batch element each). Per core
a fully fused pipeline in bf16 (fp32 accumulation):

  - Weights are host-prepped: transposed to (in, out) layout, per-head row
    mean-centering folded into wq/wk (so LayerNorm's mean subtraction becomes
    free), biases carried separately, RoPE rotation expressed as two
    elementwise tables C/S in q^T layout with the softmax scale and q-gain
    folded in.
  - All projections run on the PE with contraction on partitions:
    q^T = WqT.T @ x^T (x^T obtained via on-device bf16 cast + DMA-transpose),
    k^T similarly from y^T, v in natural layout.
  - RMS-normalization (layernorm with exact-zero mean) uses ones-block
    matmuls for the partition-axis sum of squares and a tiny selector matmul
    to broadcast rstd back across each head's 64 partitions.
  - RoPE: qr = (q*C + pairswap(q)*S) * rstd, pairswap via DVE stream_shuffle.
  - Attention per head: scores^T = k_n.T @ qr on PE (K=64), exp on ScalarE
    straight out of PSUM, attn@v with v augmented by a block of ones columns
    so the softmax denominator appears replicated in PSUM rows 64..127, then
    one reciprocal + one multiply normalizes during PSUM eviction.
  - Output projection back to natural layout with bias via a K=1 ones-row
    matmul.
"""

import os
import numpy as np
import ml_dtypes

BF = ml_dtypes.bfloat16
QDIM, KDIM, NH, HD = 1024, 2048, 16, 64
H, W, B, S2 = 64, 64, 8, 256
S1 = H * W
EPS = 1e-6
SCALE = HD ** -0.5
NQ = 4            # process s1 in quarters
S1Q = S1 // NQ    # 1024

last_exec_time_ns = None
last_trace_path = None


# ----------------------------------------------------------------- host prep
def _bf16(a):
    return np.asarray(a, np.float32).astype(BF)


def _f32(a):
    return np.asarray(a, np.float32)


def _center_rows_per_head(w):
    out = _f32(w).copy()
    for _ in range(3):
        wb = _f32(_bf16(out))
        resid = wb.reshape(NH, HD, -1).mean(axis=1, keepdims=True)
        out = wb - np.broadcast_to(resid, (NH, HD, wb.shape[-1])).reshape(wb.shape)
    return _bf16(out)


def _center_bias_per_head(b):
    b = _f32(b)
    return b - np.repeat(b.reshape(NH, HD).mean(axis=1), HD)


def _rope_tables(qn_g):
    d4 = HD // 4
    inv = 1.0 / (10000.0 ** (np.arange(0, HD // 2, 2, dtype=np.float32) / (HD // 2)))
    fh = np.outer(np.arange(H, dtype=np.float32), inv)
    fw = np.outer(np.arange(W, dtype=np.float32), inv)
    ang = np.stack([
        np.broadcast_to(fh[:, None, :], (H, W, d4)),
        np.broadcast_to(fw[None, :, :], (H, W, d4)),
    ], axis=-1).reshape(S1, HD // 2)
    cos = np.cos(ang)
    sin = np.sin(ang)
    g = _f32(qn_g)
    C = np.zeros((HD, S1), np.float32)
    S = np.zeros((HD, S1), np.float32)
    for i in range(HD // 2):
        C[2 * i] = g[2 * i] * cos[:, i] * SCALE
        C[2 * i + 1] = g[2 * i + 1] * cos[:, i] * SCALE
        S[2 * i] = -g[2 * i + 1] * sin[:, i] * SCALE
        S[2 * i + 1] = g[2 * i] * sin[:, i] * SCALE
    return _bf16(np.concatenate([C, C], 0)), _bf16(np.concatenate([S, S], 0))


def _host_tables(wq, bq, wkv, bkv, wo, bo, qn_g, qn_b, kn_g, kn_b):
    assert not np.any(_f32(qn_b)) and not np.any(_f32(kn_b)), \
        "nonzero layernorm beta not implemented"
    t = {}
    t["WqT"] = np.ascontiguousarray(_center_rows_per_head(wq).T)
    t["bq_c"] = _center_bias_per_head(bq).reshape(QDIM, 1)
    t["WkT"] = np.ascontiguousarray(_center_rows_per_head(_f32(wkv)[0:QDIM]).T)
    t["bk_c"] = _center_bias_per_head(_f32(bkv)[0:QDIM]).reshape(QDIM, 1)
    t["WvT"] = np.ascontiguousarray(_bf16(_f32(wkv)[QDIM:]).T)
    t["bv"] = _bf16(_f32(bkv)[QDIM:]).reshape(1, QDIM)
    t["WoT"] = np.ascontiguousarray(_bf16(wo).T)
    t["bo"] = _bf16(bo).reshape(1, QDIM)
    t["CT"], t["ST"] = _rope_tables(qn_g)
    t["gk_col"] = np.tile(_f32(kn_g), 2).reshape(128, 1)
    ones2 = np.zeros((128, 2), np.float32)
    ones2[0:64, 0] = 1.0
    ones2[64:128, 1] = 1.0
    t["ones2"] = _bf16(ones2)
    sel2 = np.zeros((2, 128), np.float32)
    sel2[0, 0:64] = 1.0
    sel2[1, 64:128] = 1.0
    t["sel2"] = sel2
    t["ones1"] = _bf16(np.ones((1, 128), np.float32))
    return t


# ------------------------------------------------------------- bass program
_PROGRAM = None


def _build_program(debug_taps=False):
    import concourse.bass as bass
    import concourse.bacc as bacc
    import concourse.mybir as mybir
    import concourse.tile as tile
    from contextlib import ExitStack

    bfd = mybir.dt.bfloat16
    f32d = mybir.dt.float32
    AF = mybir.ActivationFunctionType
    AO = mybir.AluOpType

    nc = bacc.Bacc("TRN2", target_bir_lowering=False, debug=False)

    def din(name, shape, dt):
        return nc.dram_tensor(name, shape, dt, kind="ExternalInput").ap()

    x_d = din("x", [S1, QDIM], f32d)
    y_d = din("y", [S2, KDIM], f32d)
    wqT_d = din("WqT", [QDIM, QDIM], bfd)
    wkT_d = din("WkT", [KDIM, QDIM], bfd)
    wvT_d = din("WvT", [KDIM, QDIM], bfd)
    woT_d = din("WoT", [QDIM, QDIM], bfd)
    bq_d = din("bq_c", [QDIM, 1], f32d)
    bk_d = din("bk_c", [QDIM, 1], f32d)
    bv_d = din("bv", [1, QDIM], bfd)
    bo_d = din("bo", [1, QDIM], bfd)
    ct_d = din("CT", [128, S1], bfd)
    st_d = din("ST", [128, S1], bfd)
    gk_d = din("gk_col", [128, 1], f32d)
    ones2_d = din("ones2", [128, 2], bfd)
    sel2_d = din("sel2", [2, 128], f32d)
    ones1_d = din("ones1", [1, 128], bfd)
    out_d = nc.dram_tensor("out", [S1, QDIM], f32d, kind="ExternalOutput").ap()
    if debug_taps:
        dbg_xt = nc.dram_tensor("dbg_xt", [128, 8, S1Q], bfd, kind="ExternalOutput").ap()
        dbg_kn = nc.dram_tensor("dbg_kn", [128, 8, S2], bfd, kind="ExternalOutput").ap()
        dbg_va = nc.dram_tensor("dbg_va", [128, 2, NH, 128], bfd, kind="ExternalOutput").ap()
        dbg_qr = nc.dram_tensor("dbg_qr", [128, 8, S1Q], bfd, kind="ExternalOutput").ap()
        dbg_aT = nc.dram_tensor("dbg_aT", [128, 8, S1Q], bfd, kind="ExternalOutput").ap()
    xbf = nc.dram_tensor("xbf", [S1, QDIM], bfd).ap()
    ybf = nc.dram_tensor("ybf", [S2, KDIM], bfd).ap()

    swap_mask = []
    for g in range(16):
        swap_mask += [2 * g + 1, 2 * g]

    with tile.TileContext(nc) as tc, ExitStack() as ctx:
        const = ctx.enter_context(tc.tile_pool(name="const", bufs=1))
        persist = ctx.enter_context(tc.tile_pool(name="persist", bufs=1))

        # ---- constants
        bq8 = const.tile([128, 8], f32d)
        nc.sync.dma_start(out=bq8[:], in_=bq_d.rearrange("(m p) o -> p (m o)", p=128))
        bk8 = const.tile([128, 8], f32d)
        nc.sync.dma_start(out=bk8[:], in_=bk_d.rearrange("(m p) o -> p (m o)", p=128))
        gk_t = const.tile([128, 1], f32d)
        nc.sync.dma_start(out=gk_t[:], in_=gk_d[:])
        bv_t = const.tile([1, QDIM], bfd)
        nc.sync.dma_start(out=bv_t[:], in_=bv_d[:])
        bo_t = const.tile([1, QDIM], bfd)
        nc.sync.dma_start(out=bo_t[:], in_=bo_d[:])
        ct_t = const.tile([128, S1], bfd)
        nc.sync.dma_start(out=ct_t[:], in_=ct_d[:])
        st_t = const.tile([128, S1], bfd)
        nc.sync.dma_start(out=st_t[:], in_=st_d[:])
        ones2_t = const.tile([128, 2], bfd)   # sum over 64-row strips
        nc.sync.dma_start(out=ones2_t[:], in_=ones2_d[:])
        sel2_t = const.tile([2, 128], f32d)   # replicate (2,n) -> (128,n)
        nc.sync.dma_start(out=sel2_t[:], in_=sel2_d[:])
        ones1_t = const.tile([1, 128], bfd)   # K=1 bias rows
        nc.sync.dma_start(out=ones1_t[:], in_=ones1_d[:])
        eps_t = const.tile([128, 1], f32d)
        nc.vector.memset(eps_t[:, :], EPS)

        # ---- persistent activations
        kn_t = persist.tile([128, 8, S2], bfd)       # normalized k^T
        va_t = persist.tile([128, 2, NH, 128], bfd)  # [v_h | ones] per s2-tile
        wq_t = persist.tile([128, 8, QDIM], bfd)
        wo_t = persist.tile([128, 8, QDIM], bfd)
        aT_t = persist.tile([128, 8, S1Q], bfd)      # per-quarter attn out^T
        qr_t = persist.tile([128, 8, S1Q], bfd)      # per-quarter roped q^T
        xt_t = persist.tile([128, 8, S1Q], bfd)      # per-quarter x^T

        # ---- casts (SWDGE, DRAM->DRAM with dtype conversion)
        nc.gpsimd.dma_start(out=ybf[:], in_=y_d[:])
        for qq in range(NQ):
            nc.gpsimd.dma_start(
                out=xbf[qq * S1Q:(qq + 1) * S1Q, :], in_=x_d[qq * S1Q:(qq + 1) * S1Q, :]
            )
        for k in range(8):
            nc.sync.dma_start(out=wq_t[:, k, :], in_=wqT_d[128 * k:128 * (k + 1), :])
        for k in range(8):
            nc.sync.dma_start(out=wo_t[:, k, :], in_=woT_d[128 * k:128 * (k + 1), :])

        # ================= KV phase =================
        with tc.tile_pool(name="kvw", bufs=1) as kvw:
            yt_t = kvw.tile([128, 16, S2], bfd)
            for k in range(16):
                nc.sync.dma_start_transpose(
                    out=yt_t[:, k, :], in_=ybf[:, 128 * k:128 * (k + 1)]
                )
            with tc.tile_pool(name="kvk", bufs=1) as kvk, \
                 tc.tile_pool(name="kv_work", bufs=2) as kvwork, \
                 tc.tile_pool(name="kv_small", bufs=2) as kvsmall, \
                 tc.tile_pool(name="psK", bufs=2, space="PSUM") as psK, \
                 tc.tile_pool(name="psSSK", bufs=2, space="PSUM") as psSSK, \
                 tc.tile_pool(name="psRepK", bufs=2, space="PSUM") as psRepK:
                wk_t = kvk.tile([128, 16, QDIM], bfd)
                for k in range(16):
                    nc.sync.dma_start(out=wk_t[:, k, :],
                                      in_=wkT_d[128 * k:128 * (k + 1), :])
                for m in range(8):
                    ps = psK.tile([128, S2], f32d, tag="psK")
                    for k in range(16):
                        nc.tensor.matmul(
                            ps[:], wk_t[:, k, 128 * m:128 * (m + 1)], yt_t[:, k, :],
                            start=(k == 0), stop=(k == 15),
                        )
                    ktb = kvwork.tile([128, S2], bfd, tag="ktb")
                    nc.scalar.activation(out=ktb[:], in_=ps[:], func=AF.Identity,
                                         bias=bk8[:, m:m + 1], scale=1.0)
                    ksq = kvwork.tile([128, S2], bfd, tag="ksq")
                    nc.scalar.activation(out=ksq[:], in_=ps[:], func=AF.Square,
                                         bias=bk8[:, m:m + 1], scale=1.0)
                    ssp = psSSK.tile([2, S2], f32d, tag="ssK")
                    nc.tensor.matmul(ssp[:], ones2_t[:], ksq[:], start=True, stop=True)
                    sd = kvsmall.tile([2, S2], f32d, tag="sdK")
                    nc.scalar.activation(out=sd[:], in_=ssp[:], func=AF.Sqrt,
                                         bias=eps_t[0:2, :], scale=1.0 / HD)
                    rstd = kvsmall.tile([2, S2], f32d, tag="rstdK")
                    nc.vector.reciprocal_approx_fast(out=rstd[:], in_=sd[:])
                    rep = psRepK.tile([128, S2], f32d, tag="repK")
                    nc.tensor.matmul(rep[:], sel2_t[:], rstd[:], start=True, stop=True)
                    nc.vector.scalar_tensor_tensor(
                        out=kn_t[:, m, :], in0=ktb[:], scalar=gk_t[:, 0:1], in1=rep[:],
                        op0=AO.mult, op1=AO.mult,
                    )

            # ---- V projection (natural layout) + v_aug build
            with tc.tile_pool(name="wv_stream", bufs=3) as wvs, \
                 tc.tile_pool(name="v_work", bufs=2) as vwork, \
                 tc.tile_pool(name="psV", bufs=1, space="PSUM") as psV:
                psvs = []
                for _mt in range(2):
                    psv = psV.tile([128, QDIM], f32d, tag=f"psV{_mt}")
                    psvs.append(psv)
                for k in range(16):
                    wv_k = wvs.tile([128, QDIM], bfd, tag="wv")
                    nc.sync.dma_start(out=wv_k[:], in_=wvT_d[128 * k:128 * (k + 1), :])
                    for mt in range(2):
                        for n in range(2):
                            sl = slice(512 * n, 512 * (n + 1))
                            nc.tensor.matmul(
                                psvs[mt][:, sl], yt_t[:, k, 128 * mt:128 * (mt + 1)],
                                wv_k[:, sl], start=(k == 0), stop=False,
                            )
                for mt in range(2):
                    for n in range(2):
                        sl = slice(512 * n, 512 * (n + 1))
                        nc.tensor.matmul(psvs[mt][:, sl], ones1_t[:], bv_t[:, sl],
                                         start=False, stop=True)
                    vbf = vwork.tile([128, QDIM], bfd, tag="vbf")
                    nc.scalar.activation(out=vbf[:], in_=psvs[mt][:], func=AF.Copy)
                    nc.vector.tensor_copy(
                        out=va_t[:, mt, :, 0:64],
                        in_=vbf.rearrange("p (h d) -> p h d", h=NH),
                    )
                    nc.vector.memset(va_t[:, mt, :, 64:128], 1.0)

        # ================= per-quarter main pipeline =================
        for qq in range(NQ):
            qoff = qq * S1Q
            for k in range(8):
                nc.sync.dma_start_transpose(
                    out=xt_t[:, k, :], in_=xbf[qoff:qoff + S1Q, 128 * k:128 * (k + 1)]
                )
            # ---- q projection + RMS-norm + RoPE
            with tc.tile_pool(name="q_work", bufs=3) as qwork, \
                 tc.tile_pool(name="q_small", bufs=3) as qsmall, \
                 tc.tile_pool(name="psQ", bufs=2, space="PSUM") as psQ, \
                 tc.tile_pool(name="psSS", bufs=2, space="PSUM") as psSS, \
                 tc.tile_pool(name="psRep", bufs=2, space="PSUM") as psRep:
                for m in range(8):
                    qt = qwork.tile([128, S1Q], bfd, tag="qt")
                    reps = []
                    for n in range(2):
                        sl = slice(512 * n, 512 * (n + 1))
                        psq = psQ.tile([128, 512], f32d, tag="psQ")
                        for k in range(8):
                            nc.tensor.matmul(
                                psq[:], wq_t[:, k, 128 * m:128 * (m + 1)],
                                xt_t[:, k, sl], start=(k == 0), stop=(k == 7),
                            )
                        nc.scalar.activation(out=qt[:, sl], in_=psq[:], func=AF.Identity,
                                             bias=bq8[:, m:m + 1], scale=1.0)
                        q2 = qwork.tile([128, 512], bfd, tag="q2")
                        nc.scalar.activation(out=q2[:], in_=psq[:], func=AF.Square,
                                             bias=bq8[:, m:m + 1], scale=1.0)
                        ssp = psSS.tile([2, 512], f32d, tag="ssQ")
                        nc.tensor.matmul(ssp[:], ones2_t[:], q2[:], start=True, stop=True)
                        sd = qsmall.tile([2, 512], f32d, tag="sdQ")
                        nc.scalar.activation(out=sd[:], in_=ssp[:], func=AF.Sqrt,
                                             bias=eps_t[0:2, :], scale=1.0 / HD)
                        rstd = qsmall.tile([2, 512], f32d, tag="rstdQ")
                        nc.vector.reciprocal_approx_fast(out=rstd[:], in_=sd[:])
                        rep = psRep.tile([128, 512], f32d, tag="repQ")
                        nc.tensor.matmul(rep[:], sel2_t[:], rstd[:], start=True, stop=True)
                        reps.append(rep)
                    # rope on full (128, S1Q) rows, rstd applied per 512-chunk
                    qs = qwork.tile([128, S1Q], bfd, tag="qs")
                    nc.vector.stream_shuffle(out=qs[:], in_=qt[:], mask=swap_mask)
                    t1 = qwork.tile([128, S1Q], bfd, tag="t1")
                    nc.vector.tensor_mul(t1[:], qt[:], ct_t[:, qoff:qoff + S1Q])
                    t2 = qwork.tile([128, S1Q], bfd, tag="t2")
                    nc.vector.tensor_mul(t2[:], qs[:], st_t[:, qoff:qoff + S1Q])
                    core = qwork.tile([128, S1Q], bfd, tag="core")
                    nc.vector.tensor_add(core[:], t1[:], t2[:])
                    for n in range(2):
                        sl = slice(512 * n, 512 * (n + 1))
                        nc.vector.tensor_mul(qr_t[:, m, sl], core[:, sl], reps[n][:])

            # ---- attention per head
            with tc.tile_pool(name="a_work", bufs=4) as awork, \
                 tc.tile_pool(name="a_rcp", bufs=2) as arcp, \
                 tc.tile_pool(name="psSC", bufs=2, space="PSUM") as psSC, \
                 tc.tile_pool(name="psAV", bufs=1, space="PSUM") as psAV:
                for h in range(NH):
                    m2, strip = h // 2, (h % 2) * 64
                    rs = slice(strip, strip + 64)
                    exps = []
                    for s2t in range(2):
                        psc = psSC.tile([128, S1Q], f32d, tag="psSC")
                        for n in range(2):
                            sl = slice(512 * n, 512 * (n + 1))
                            nc.tensor.matmul(
                                psc[:, sl],
                                kn_t[rs, m2, 128 * s2t:128 * (s2t + 1)],
                                qr_t[rs, m2, sl], start=True, stop=True,
                            )
                        ex = awork.tile([128, S1Q], bfd, tag="ex")
                        nc.scalar.activation(out=ex[:], in_=psc[:], func=AF.Exp)
                        exps.append(ex)
                    # value rows and denominator rows both land on partitions
                    # [strip, strip+64) so the normalize stays lane-aligned
                    pav = psAV.tile([128, S1Q], f32d, tag="psAV")
                    pad = psAV.tile([128, S1Q], f32d, tag="psAD")
                    for k in range(2):
                        for n in range(2):
                            sl = slice(512 * n, 512 * (n + 1))
                            nc.tensor.matmul(
                                pav[rs, sl], va_t[:, k, h, 0:64], exps[k][:, sl],
                                start=(k == 0), stop=(k == 1),
                                tile_position=(0, strip),
                            )
                            nc.tensor.matmul(
                                pad[rs, sl], va_t[:, k, h, 64:128], exps[k][:, sl],
                                start=(k == 0), stop=(k == 1),
                                tile_position=(0, strip),
                            )
                    rcp = arcp.tile([128, S1Q], f32d, tag="rcp")
                    nc.vector.reciprocal_approx_fast(out=rcp[:, :], in_=pad[:, :])
                    nc.vector.tensor_mul(aT_t[rs, m2, :], pav[rs, :], rcp[rs, :])

            if debug_taps and qq == 0:
                nc.sync.dma_start(out=dbg_xt[:], in_=xt_t[:])
                nc.sync.dma_start(out=dbg_kn[:], in_=kn_t[:])
                nc.sync.dma_start(out=dbg_va[:], in_=va_t[:])
                nc.sync.dma_start(out=dbg_qr[:], in_=qr_t[:])
                nc.sync.dma_start(out=dbg_aT[:], in_=aT_t[:])
            # ---- output projection (natural layout)
            with tc.tile_pool(name="o_work", bufs=2) as owork, \
                 tc.tile_pool(name="psO", bufs=2, space="PSUM") as psO:
                for mo in range(8):
                    pso = psO.tile([128, QDIM], f32d, tag="psO")
                    for n in range(2):
                        sl = slice(512 * n, 512 * (n + 1))
                        for k in range(8):
                            nc.tensor.matmul(
                                pso[:, sl], aT_t[:, k, 128 * mo:128 * (mo + 1)],
                                wo_t[:, k, sl], start=(k == 0), stop=False,
                            )
                        nc.tensor.matmul(pso[:, sl], ones1_t[:], bo_t[:, sl],
                                         start=False, stop=True)
                    osb = owork.tile([128, QDIM], f32d, tag="osb")
                    nc.scalar.activation(out=osb[:], in_=pso[:], func=AF.Copy)
                    nc.sync.dma_start(
                        out=out_d[qoff + 128 * mo:qoff + 128 * (mo + 1), :], in_=osb[:]
                    )

    nc.compile()
    return nc


def _get_program():
    global _PROGRAM
    if _PROGRAM is None:
        _PROGRAM = _build_program()
    return _PROGRAM


# ------------------------------------------------------------------- kernel
def kernel(x, y, wq, bq, wkv, bkv, wo, bo, qn_g, qn_b, kn_g, kn_b):
    global last_exec_time_ns, last_trace_path
    from concourse.bass_utils import run_bass_kernel_spmd

    t = _host_tables(wq, bq, wkv, bkv, wo, bo, qn_g, qn_b, kn_g, kn_b)
    x = _f32(x)
    y = _f32(y)
    nc = _get_program()
    in_maps = []
    for c in range(B):
        m = dict(t)
        m["x"] = np.ascontiguousarray(x[c])
        m["y"] = np.ascontiguousarray(y[c])
        in_maps.append(m)
    trace = bool(int(os.environ.get("KERNEL_TRACE", "0")))
    res = run_bass_kernel_spmd(nc, in_maps, core_ids=list(range(B)), trace=trace)
    last_exec_time_ns = res.exec_time_ns
    if res.instructions_and_trace is not None:
        last_trace_path = res.instructions_and_trace[1]
    return np.stack([res.results[c]["out"] for c in range(B)]).astype(np.float32)

